# revision 47
# baseline (speedup 1.0000x reference)
"""Trainium2 Bass kernel for nn_LinearTransformer (linear attention, 4 layers x 8 heads).

Math: each layer computes Z += sum_j (Z Qf_j Z^T)(mask . Z Pf_j^T)/(N-1), which
factorizes exactly (linear attention):
    Z_{l+1} = Z_l (I + A_l),   A_l = sum_j Qf_j G'_l Pf_j^T / (N-1)
    G'_l = Z_l^T Z_l - z_l z_l^T   (z_l = last token row)
Right-multiplicative layers collapse: Z_l = Z_0 C_l, and with
H_l = C_l^T G'_0 C_l (symmetric), D_l = C_l^T:
    U_l   = H_l @ PTs_l                     (PTs = scaled P_full^T blocks)
    A_l   = sum_j Qf_j U_{l,j}              (PSUM accumulation)
    IA    = I + A_l
    H_l+1 = IA^T (H_l IA)                   (two matmuls, H stays symmetric)
    D_l+1 = IA^T D_l
    Z_out = Z_0 C_4 = Z_0 D_4^T
The device streams Z only twice (Gram + final product); everything else is 64x64.

All device compute runs in bf16 (inputs cast on host; PSUM accumulates
fp32; f32 output): matmuls run at 1 cycle/row instead of fp32's 4, DVE/ACT
copies hit 2x mode, and input DMA bytes halve. Measured rel err 4.6e-3
against the f32 reference (tolerance gate 2e-2).

Sharding: data-parallel over batch B=16 across 8 cores (2 batches/core, no
collectives). Middle recurrence runs as two engine-parallel chains (batch 0
copies on DVE, batch 1 on ACT).
"""

import os
import numpy as np

B, N, D = 16, 2048, 64
NL, NH, DP = 4, 8, 63
NCORES = 8
BPC = B // NCORES  # 2 batches per core
NCHUNK = N // 128  # 16
NQ = 4  # DMA quarters
CPQ = NCHUNK // NQ  # chunks per quarter
SCALE = 1.0 / (N - 1)

_cache = {}


def _build():
    import concourse.bass as bass
    import concourse.mybir as mybir
    import concourse.tile as tile
    from concourse import bacc
    from concourse.masks import make_identity

    f32 = mybir.dt.float32
    bf16 = mybir.dt.bfloat16

    nc = bacc.Bacc(
        "TRN2",
        target_bir_lowering=False,
        debug=False,
        enable_asserts=True,
        num_devices=NCORES,
    )

    Zd = nc.dram_tensor("Z", [BPC, N, D], bf16, kind="ExternalInput")
    PTd = nc.dram_tensor("PT", [D, NL, 512], bf16, kind="ExternalInput")
    QTd = nc.dram_tensor("QT", [D, NL, 512], bf16, kind="ExternalInput")
    Od = nc.dram_tensor("O", [BPC, N, D], f32, kind="ExternalOutput")

    with tile.TileContext(nc) as tc:
        with (
            tc.tile_pool(name="const", bufs=1) as const,
            tc.tile_pool(name="zbuf", bufs=1) as zbuf,
            tc.tile_pool(name="mid", bufs=3) as mid,
            tc.tile_pool(name="pbig", bufs=2, space="PSUM") as pbig,
            tc.tile_pool(name="pacc", bufs=1, space="PSUM") as pacc,
            tc.tile_pool(name="pmix", bufs=4, space="PSUM") as pmix,
        ):
            ident = const.tile([128, 128], bf16)
            make_identity(nc, ident)
            i64 = ident[0:64, 0:64]
            identf = const.tile([64, 64], f32)
            make_identity(nc, identf)
            # engine warm-ups during the DMA dead time: start the PE clock
            # ramp and pull ACT's LoadActFuncSet off the first chain copy
            pwarm = pmix.tile([128, 64], f32, tag="mid", name="pwarm")
            nc.tensor.matmul(
                pwarm, lhsT=ident, rhs=ident[:, 0:64], start=True, stop=True
            )
            awarm = const.tile([64, 64], f32)
            nc.scalar.copy(awarm, identf)

            # last-token rows at partition 0 (rank-1 Gram correction), then Z
            # quarters on the SP queue; params on the gpsimd queue in parallel.
            zslab = const.tile([1, BPC, D], bf16)
            ztq = []
            for q in range(NQ):
                zt = zbuf.tile([128, CPQ, BPC, D], bf16, tag=f"zt{q}", name=f"zt{q}")
                ztq.append(zt)
                if q == 0:
                    # chunk 0 lands first so PE starts early
                    nc.sync.dma_start(
                        out=zt[:, 0, :, :], in_=Zd[:, 0:128, :].rearrange("b t d -> t b d")
                    )
                    for b in range(BPC):
                        nc.sync.dma_start(
                            out=zt[:, 1:, b, :],
                            in_=Zd[b, 128 : CPQ * 128, :].rearrange(
                                "(c t) d -> t c d", t=128
                            ),
                        )
                    nc.sync.dma_start(
                        out=zslab, in_=Zd[:, N - 1 : N, :].rearrange("b t d -> t b d")
                    )
                else:
                    qeng = {1: nc.sync, 2: nc.sync, 3: nc.sync}[q]
                    for b in range(BPC):
                        qeng.dma_start(
                            out=zt[:, :, b, :],
                            in_=Zd[b, q * CPQ * 128 : (q + 1) * CPQ * 128, :].rearrange(
                                "(c t) d -> t c d", t=128
                            ),
                        )
                if q == 0:
                    PTs = const.tile([D, NL, 512], bf16)
                    nc.gpsimd.dma_start(out=PTs, in_=PTd[:, :, :])
                    QTs = const.tile([D, NL, 512], bf16)
                    nc.gpsimd.dma_start(out=QTs, in_=QTd[:, :, :])

            negz = const.tile([1, BPC, D], bf16)
            nc.vector.tensor_scalar_mul(negz, zslab, -1.0)

            # --- phase 1: Gram matrices (per batch, all base-0) + transposes ---
            Wstack = zbuf.tile([128, N], bf16)  # [(b,d), token]
            pg = [pacc.tile([64, 64], f32, tag=f"pg{b}", name=f"pg{b}") for b in range(BPC)]
            for c in range(NCHUNK):
                zt = ztq[c // CPQ]
                cc = c % CPQ
                Zc = zt[:, cc, :, :].rearrange("p b d -> p (b d)")
                if c % 2 == 0:
                    pw = pbig.tile([128, 2, 128], bf16, tag="big")
                    nc.tensor.transpose(pw[:, 0, :], Zc, ident)
                else:
                    nc.tensor.transpose(pw[:, 1, :], Zc, ident)
                for b in range(BPC):
                    nc.tensor.matmul(
                        pg[b],
                        lhsT=zt[:, cc, b, :],
                        rhs=zt[:, cc, b, :],
                        start=(c == 0),
                        stop=False,
                    )
                if c % 2 == 1:
                    eng = nc.vector if (c // 2) % 2 == 0 else nc.scalar
                    (eng.tensor_copy if eng is nc.vector else eng.copy)(
                        Wstack[:, (c - 1) * 128 : (c + 1) * 128],
                        pw.rearrange("p k a -> p (k a)"),
                    )
            # G -= z z^T
            Hs = [None, None]
            for b in range(BPC):
                nc.tensor.matmul(
                    pg[b],
                    lhsT=negz[0:1, b, :],
                    rhs=zslab[0:1, b, :],
                    start=False,
                    stop=True,
                )
            g0 = mid.tile([64, D], bf16, tag="h0")
            nc.vector.tensor_copy(g0, pg[0])
            g1 = mid.tile([64, D], bf16, tag="h1")
            nc.scalar.copy(g1, pg[1])
            Hs = [g0, g1]

            # --- middle recurrence: two engine-parallel chains ---
            cp = [
                lambda o, i: nc.vector.tensor_copy(o, i),
                lambda o, i: nc.scalar.copy(o, i),
            ]
            Ds = [None, None]
            for l in range(NL):
                pU, Us, pA, IAs, pR, Rs, pD, pH = (
                    [None] * 2, [None] * 2, [None] * 2, [None] * 2,
                    [None] * 2, [None] * 2, [None] * 2, [None] * 2,
                )
                for b in range(BPC):
                    pU[b] = pmix.tile([64, 512], f32, tag="mid", name=f"pU{b}_{l}")
                    nc.tensor.matmul(
                        pU[b], lhsT=Hs[b], rhs=PTs[:, l, :], start=True, stop=True
                    )
                for b in range(BPC):
                    Us[b] = mid.tile([64, 512], bf16, tag=f"us{b}", name=f"us{b}_{l}")
                    cp[b](Us[b], pU[b])
                for b in range(BPC):
                    pA[b] = pmix.tile([64, 64], f32, tag="mid", name=f"pA{b}_{l}")
                    for j in range(NH):
                        nc.tensor.matmul(
                            pA[b],
                            lhsT=QTs[:, l, j * 64 : (j + 1) * 64],
                            rhs=Us[b][:, j * 64 : (j + 1) * 64],
                            start=(j == 0),
                            stop=(j == NH - 1),
                        )
                for b in range(BPC):
                    # IA = I + A, fused into the PSUM drain (ACT cannot do
                    # tensor+tensor, so both adds ride DVE)
                    IAs[b] = mid.tile([64, D], bf16, tag=f"ia{b}", name=f"ia{b}_{l}")
                    nc.vector.tensor_add(IAs[b], identf, pA[b])
                # PE: R (skip last layer), D updates
                if l < NL - 1:
                    for b in range(BPC):
                        pR[b] = pmix.tile([64, D], f32, tag="mid", name=f"pR{b}_{l}")
                        nc.tensor.matmul(
                            pR[b], lhsT=Hs[b], rhs=IAs[b], start=True, stop=True
                        )
                for b in range(BPC):
                    pD[b] = pmix.tile([64, D], f32, tag="mid", name=f"pD{b}_{l}")
                    nc.tensor.matmul(
                        pD[b],
                        lhsT=IAs[b],
                        rhs=(Ds[b] if l > 0 else i64),
                        start=True,
                        stop=True,
                    )
                if l < NL - 1:
                    for b in range(BPC):
                        Rs[b] = mid.tile([64, D], bf16, tag=f"rs{b}", name=f"rs{b}_{l}")
                        cp[b](Rs[b], pR[b])
                for b in range(BPC):
                    Ds[b] = mid.tile([64, D], bf16, tag=f"ds{b}", name=f"ds{b}_{l}")
                    cp[b](Ds[b], pD[b])
                if l < NL - 1:
                    for b in range(BPC):
                        pH[b] = pmix.tile([64, D], f32, tag="mid", name=f"pH{b}_{l}")
                        nc.tensor.matmul(
                            pH[b], lhsT=IAs[b], rhs=Rs[b], start=True, stop=True
                        )
                    for b in range(BPC):
                        Hs[b] = mid.tile([64, D], bf16, tag=f"h{b}", name=f"hn{b}_{l}")
                        cp[b](Hs[b], pH[b])

            # --- C4 = D4^T per batch, assembled block-diagonally ---
            pce = pmix.tile([128, D], f32, tag="mid")
            nc.tensor.matmul(pce[0:64, :], lhsT=Ds[0], rhs=i64, start=True, stop=True)
            nc.tensor.matmul(pce[64:128, :], lhsT=Ds[1], rhs=i64, start=True, stop=True)
            C4blk = mid.tile([128, BPC * D], bf16, tag="c4")
            nc.gpsimd.memset(C4blk, 0.0)
            nc.vector.tensor_copy(C4blk[0:64, 0:D], pce[0:64, :])
            nc.scalar.copy(C4blk[64:128, D : 2 * D], pce[64:128, :])

            # --- Z_out = Z C4, streamed back by quarters ---
            for q in range(NQ):
                zo = zbuf.tile([128, CPQ, BPC, D], f32, tag=f"zo{q}", name=f"zo{q}")
                for c2 in range(CPQ // 2):
                    po = pbig.tile([128, 2, BPC * D], f32, tag="big")
                    for k in range(2):
                        c = q * CPQ + 2 * c2 + k
                        nc.tensor.matmul(
                            po[:, k, :],
                            lhsT=Wstack[:, c * 128 : (c + 1) * 128],
                            rhs=C4blk,
                            start=True,
                            stop=True,
                        )
                    eng_i = (q * (CPQ // 2) + c2) % 2
                    if eng_i == 0:
                        nc.vector.tensor_copy(
                            zo[:, 2 * c2 : 2 * c2 + 2, :, :],
                            po.rearrange("t k (b d) -> t k b d", b=BPC),
                        )
                    else:
                        nc.scalar.copy(
                            zo[:, 2 * c2 : 2 * c2 + 2, :, :],
                            po.rearrange("t k (b d) -> t k b d", b=BPC),
                        )
                for b in range(BPC):
                    nc.sync.dma_start(
                        out=Od[b, q * CPQ * 128 : (q + 1) * CPQ * 128, :].rearrange(
                            "(c t) d -> t c d", t=128
                        ),
                        in_=zo[:, :, b, :],
                    )

    nc.compile()
    return nc


def _get_nc():
    if "nc" not in _cache:
        _cache["nc"] = _build()
    return _cache["nc"]


def _host_params(allparam):
    ap = np.asarray(allparam, dtype=np.float32)
    Pf = np.zeros((NL, NH, D, D), np.float32)
    Qf = np.zeros((NL, NH, D, D), np.float32)
    Pf[:, :, :DP, :DP] = ap[:, :, 0]
    Pf[:, :, DP, DP] = 1.0
    Qf[:, :, :DP, :DP] = ap[:, :, 1]
    # PT[d, l, j*64+e] = Pf[l,j,e,d] * SCALE  (P_full^T blocks side by side)
    import ml_dtypes

    PT = np.ascontiguousarray(
        (Pf.transpose(3, 0, 1, 2) * SCALE).reshape(D, NL, NH * D)
    ).astype(ml_dtypes.bfloat16)
    QT = np.ascontiguousarray(
        Qf.transpose(3, 0, 1, 2).reshape(D, NL, NH * D)
    ).astype(ml_dtypes.bfloat16)
    return PT, QT


def kernel(Z, allparam):
    import ml_dtypes
    from concourse.bass_utils import run_bass_kernel_spmd

    Z = np.asarray(Z, dtype=np.float32).astype(ml_dtypes.bfloat16)
    PT, QT = _host_params(allparam)
    nc = _get_nc()

    in_maps = []
    for core in range(NCORES):
        zshard = np.ascontiguousarray(Z[core * BPC : (core + 1) * BPC])
        in_maps.append({"Z": zshard, "PT": PT, "QT": QT})

    res = run_bass_kernel_spmd(
        nc,
        in_maps,
        core_ids=list(range(NCORES)),
        trace=bool(int(os.environ.get("KERNEL_TRACE", "0") or "0")),
    )
    _cache["last_results"] = res

    out = np.empty((B, N, D), np.float32)
    for core in range(NCORES):
        out[core * BPC : (core + 1) * BPC] = res.results[core]["O"]
    return out

